# revision 1
# baseline (speedup 1.0000x reference)
"""Bass/Trainium2 kernel for nn_CustomBBoxLoss.

Reference computation:
    A1 = pred.sum(axis=(0,1));  A2 = (pred**2).sum(axis=(0,1))      # [H, W]
    s1[b] = sum of A1 over box b's region;  s2[b] likewise for A2
    per_box = (s2 - 2*cls*s1 + cls^2*cnt) / cnt;  loss = per_box.mean()

Instead of integral images, each region sum is a bilinear form
    s[b] = rowmask_b^T @ A @ colmask_b
with 0/1 interval masks.  The row-mask contraction (fused with the
(B,C)-map reduction) runs on the PE as matmuls with the transposed row
masks as stationary operands; the col-mask contraction is a fused
multiply-reduce on the vector engine.

Sharding: pred's [2048, 2048] spatial plane is split into a 4x2 grid
(512 rows x 1024 cols per core) over 8 cores.  Box index vectors are
replicated (shifted by each core's origin so the same SPMD program works
on every core).  Each core emits per-box partial sums; the host sums the
8 partials (the "all-reduce") and applies the closed-form per-box
formula to produce the scalar loss.
"""

import numpy as np

H = W = 2048
B, C, N = 2, 3, 256
MAPS = B * C                      # 6
RB, CB = 4, 2                     # row-blocks x col-blocks = 8 cores
ROWS, COLS = H // RB, W // CB     # 512 x 1024 per core
P = 128                           # partitions
NRT = ROWS // P                   # 4 row tiles per core
NPAIR = MAPS // 2                 # 3 map pairs
NK = NRT * NPAIR                  # 12 streamed tiles per core
NCH = COLS // 512                 # 2 col chunks of 512
NB = N // P                       # 2 box halves

_CACHE = {}


def _build_module():
    import concourse.bacc as bacc
    import concourse.mybir as mybir
    import concourse.tile as tile

    f32 = mybir.dt.float32
    bf16 = mybir.dt.bfloat16
    i32 = mybir.dt.int32
    Alu = mybir.AluOpType

    nc = bacc.Bacc("TRN2", target_bir_lowering=False, debug=False)

    pred_part = nc.declare_dram_parameter("pred_part", [NK, P, 2048], f32, isOutput=False)
    # index vectors staged host-side: broadcast to [P, N] f32 (values exact)
    ybs = nc.declare_dram_parameter("ybs", [P, N], f32, isOutput=False)
    hb = nc.declare_dram_parameter("hb", [P, N], f32, isOutput=False)
    # packed col-mask inputs: columns (x_b0, x_b1, w_b0, w_b1)
    xw = nc.declare_dram_parameter("xw", [P, 4], f32, isOutput=False)
    # columns: (b*2 + q) -> partial region sums for boxes [b*128, (b+1)*128)
    out_s = nc.declare_dram_parameter("out_s", [P, NB * 2], f32, isOutput=True)

    with tile.TileContext(nc) as tc:
        with (
            tc.tile_pool(name="persist", bufs=1) as pp,
            tc.tile_pool(name="stage", bufs=8) as stage_pool,
            tc.tile_pool(name="t1p", bufs=3) as t1_pool,
            tc.tile_pool(name="sqp", bufs=6) as sq_pool,
            tc.tile_pool(name="scratch", bufs=2) as scr_pool,
            tc.tile_pool(name="psum", bufs=1, space="PSUM") as psum_pool,
        ):
            # ---- small box-vector DMAs first: they finish fast and unblock
            # the mask chain; behind the 1MB transfers they'd take ~7us ----
            ybc = pp.tile([P, N], f32, tag="ybc", name="ybc")
            hbc = pp.tile([P, N], f32, tag="hbc", name="hbc")
            xw_t = pp.tile([P, 4], f32, tag="xw_t", name="xw_t")
            nc.sync.dma_start(ybc[:], ybs.ap()[:])
            nc.sync.dma_start(hbc[:], hb.ap()[:])
            nc.sync.dma_start(xw_t[:], xw.ap()[:])
            xcol = [xw_t[:, b:b + 1] for b in range(NB)]

            # ---- big streaming DMAs (issue order on sync = program order);
            # first two tiles split in half across queues to cut the latency
            # of the pipeline-filling transfers ----
            stages = []
            for k in range(NK):
                stage = stage_pool.tile([P, 2048], f32, tag="stage", name="stage")
                if k < 2:
                    nc.sync.dma_start(stage[:, :COLS], pred_part.ap()[k][:, :COLS])
                    nc.sync.dma_start(stage[:, COLS:], pred_part.ap()[k][:, COLS:])
                else:
                    nc.sync.dma_start(stage[:], pred_part.ap()[k])
                stages.append(stage)

            # ---- gpsimd: iotas + tiny adds ----
            rvecs = []
            for rt in range(NRT):
                rvec = pp.tile([P, 1], f32, tag=f"rvec{rt}", name=f"rvec{rt}")
                nc.gpsimd.iota(rvec[:], pattern=[[0, 1]], base=rt * P,
                               channel_multiplier=1,
                               allow_small_or_imprecise_dtypes=True)
                rvecs.append(rvec)
            y2bc = pp.tile([P, N], f32, tag="y2bc", name="y2bc")
            nc.gpsimd.tensor_add(y2bc[:], ybc[:], hbc[:])
            # iota_c after the row-mask inputs: only needed for col masks
            iota_c = pp.tile([P, COLS], f32, tag="iota_c", name="iota_c")
            nc.gpsimd.iota(iota_c[:], pattern=[[1, COLS]], base=0,
                           channel_multiplier=0,
                           allow_small_or_imprecise_dtypes=True)
            x2_t = pp.tile([P, 2], f32, tag="x2_t", name="x2_t")
            nc.gpsimd.tensor_add(x2_t[:], xw_t[:, 0:2], xw_t[:, 2:4])
            x2col = [x2_t[:, b:b + 1] for b in range(NB)]

            # ---- DVE mask work, emitted lazily between pair-adds ----
            rmaskT = [None] * NRT
            rmask_mul = [None] * NRT

            def build_rmask(rt):
                ge = pp.tile([P, N], bf16, tag=f"rge{rt}", name=f"rge{rt}")
                lt = pp.tile([P, N], bf16, tag=f"rlt{rt}", name=f"rlt{rt}")
                nc.vector.tensor_scalar(ge[:], ybc[:], rvecs[rt][:], None, Alu.is_le)
                nc.vector.tensor_scalar(lt[:], y2bc[:], rvecs[rt][:], None, Alu.is_gt)
                rm = pp.tile([P, N], bf16, tag=f"rmask{rt}", name=f"rmask{rt}")
                rmask_mul[rt] = nc.vector.tensor_mul(rm[:], ge[:], lt[:])
                rmaskT[rt] = rm

            cmask = []
            cm_parts = []
            for b in range(NB):
                cge = pp.tile([P, COLS], f32, tag=f"cge{b}", name=f"cge{b}")
                clt = pp.tile([P, COLS], f32, tag=f"clt{b}", name=f"clt{b}")
                cm = pp.tile([P, COLS], f32, tag=f"cmask{b}", name=f"cmask{b}")
                cmask.append(cm)
                cm_parts.append((cge, clt, cm))

            # one deferred DVE op per call, spread across stream iterations
            cmask_ops = []
            for b in range(NB):
                cge, clt, cm = cm_parts[b]
                cmask_ops.append(lambda b=b, cge=cge: nc.vector.tensor_scalar(
                    cge[:], iota_c[:], xcol[b][:], None, Alu.is_ge))
                cmask_ops.append(lambda b=b, clt=clt: nc.vector.tensor_scalar(
                    clt[:], iota_c[:], x2col[b][:], None, Alu.is_lt))
                cmask_ops.append(lambda cge=cge, clt=clt, cm=cm: nc.vector.tensor_mul(
                    cm[:], cge[:], clt[:]))

            # ---- PSUM groups: (box half, quantity, col chunk) ----
            ps = {}
            for b in range(NB):
                for q in range(2):
                    for n in range(NCH):
                        ps[(b, q, n)] = psum_pool.tile(
                            [P, 512], f32, tag=f"ps{b}{q}{n}", name=f"ps{b}{q}{n}")

            # ---- PE prewarm: ~3.5us of junk matmuls while DMAs fill, so the
            # HAM clock gate is at 2.4GHz when the real stream begins ----
            junk = pp.tile([P, 512], bf16, tag="junk", name="junk")
            nc.gpsimd.memset(junk[:], 0.0)
            for w in range(16):
                nc.tensor.matmul(ps[(0, 0, 0)][:], junk[:, :P], junk[:],
                                 start=True, stop=True)

            # s_all columns: b*4 + q*2 + n
            s_all = pp.tile([P, 8], f32, tag="s_all", name="s_all")

            # ---- stream: pair-add (T1) + square (T2) + matmuls ----
            for rt in range(NRT):
                for j in range(NPAIR):
                    k = rt * NPAIR + j
                    stage = stages[k]

                    # lazily emit DVE mask work just-in-time, between pair-adds
                    if j == 0 and rmaskT[rt] is None:
                        build_rmask(rt)
                    t1t = t1_pool.tile([P, COLS], bf16, tag="t1t", name="t1t")
                    pa = nc.vector.tensor_add(t1t[:], stage[:, :COLS], stage[:, COLS:])
                    if k == 0:
                        # force DVE order: row-mask 0 before the first pair-add,
                        # so the first matmul isn't gated on a late mask build
                        from concourse.tile_rust import add_dep_helper
                        add_dep_helper(pa.ins, rmask_mul[0].ins, sync=False,
                                       reason="rmask0 before first pair-add")
                    if j == 0 and rt + 1 < NRT and rmaskT[rt + 1] is None:
                        build_rmask(rt + 1)   # stay one row-tile ahead
                    if k >= 1 and cmask_ops:
                        cmask_ops.pop(0)()
                        if k >= 4 and cmask_ops:
                            cmask_ops.pop(0)()
                    sq0 = sq_pool.tile([P, COLS], bf16, tag="sq0", name="sq0")
                    sq1 = sq_pool.tile([P, COLS], bf16, tag="sq1", name="sq1")
                    nc.scalar.square(sq0[:], stage[:, :COLS])
                    nc.scalar.square(sq1[:], stage[:, COLS:])

                    for b in range(NB):
                        lhsT = rmaskT[rt][:, b * P:(b + 1) * P]
                        for n in range(NCH):
                            nc.tensor.matmul(
                                ps[(b, 0, n)][:], lhsT, t1t[:, n * 512:(n + 1) * 512],
                                start=(k == 0), stop=(k == NK - 1))
                        for si, sqt in ((0, sq0), (1, sq1)):
                            for n in range(NCH):
                                nc.tensor.matmul(
                                    ps[(b, 1, n)][:], lhsT, sqt[:, n * 512:(n + 1) * 512],
                                    start=(2 * k + si == 0),
                                    stop=(2 * k + si == 2 * NK - 1))

            # ---- col-mask apply (order matches PE group completion order) ----
            for b in range(NB):
                for q in range(2):
                    for n in range(NCH):
                        g = b * 4 + q * 2 + n
                        scr = scr_pool.tile([P, 512], f32, tag="scr", name="scr")
                        nc.vector.scalar_tensor_tensor(
                            out=scr[:],
                            in0=ps[(b, q, n)][:],
                            scalar=1.0,
                            in1=cmask[b][:, n * 512:(n + 1) * 512],
                            op0=Alu.mult,
                            op1=Alu.mult,
                            accum_out=s_all[:, g:g + 1],
                        )

            # ---- merge col chunks: 1 DVE op, single out DMA ----
            sfin = pp.tile([P, NB * 2], f32, tag="sfin", name="sfin")
            nc.vector.tensor_reduce(
                sfin[:], s_all[:].rearrange("p (g n) -> p g n", n=NCH),
                mybir.AxisListType.X, Alu.add)
            nc.gpsimd.dma_start(out_s.ap()[:], sfin[:])

    nc.compile()
    return nc


def _get_module():
    if "nc" not in _CACHE:
        _CACHE["nc"] = _build_module()
    return _CACHE["nc"]


def _make_in_maps(pred, box_y, box_x, box_h, box_w):
    pred6 = np.ascontiguousarray(pred).reshape(MAPS, H, W)
    in_maps = []
    for core in range(RB * CB):
        rb, cb = divmod(core, CB)
        slab = pred6[:, rb * ROWS:(rb + 1) * ROWS, cb * COLS:(cb + 1) * COLS]
        a = slab.reshape(NPAIR, 2, NRT, P, COLS)       # [j, t, rt, p, c]
        a = np.ascontiguousarray(a.transpose(2, 0, 3, 1, 4))  # [rt, j, p, t, c]
        ybsf = (box_y - rb * ROWS).astype(np.float32)
        xbsf = (box_x - cb * COLS).astype(np.float32)
        wbf = box_w.astype(np.float32)
        in_maps.append({
            "pred_part": a.reshape(NK, P, 2048),
            "ybs": np.ascontiguousarray(np.broadcast_to(ybsf, (P, N))),
            "hb": np.ascontiguousarray(
                np.broadcast_to(box_h.astype(np.float32), (P, N))),
            "xw": np.stack([xbsf[:P], xbsf[P:], wbf[:P], wbf[P:]], axis=1),
        })
    return in_maps


def _finalize(results, box_h, box_w, box_cls):
    s1 = np.zeros(N, np.float64)
    s2 = np.zeros(N, np.float64)
    for r in results:
        o = r["out_s"].astype(np.float64)  # [128, (b, q)]
        for b in range(NB):
            s1[b * P:(b + 1) * P] += o[:, b * 2 + 0]
            s2[b * P:(b + 1) * P] += o[:, b * 2 + 1]
    cnt = float(MAPS) * (box_h.astype(np.float64) * box_w.astype(np.float64))
    cls = box_cls.astype(np.float64)
    per_box = (s2 - 2.0 * cls * s1 + cls * cls * cnt) / cnt
    return np.asarray(per_box.mean(), dtype=np.float32)


def kernel(pred, box_y, box_x, box_h, box_w, box_cls, _bench=None):
    from concourse.bass_utils import run_bass_kernel_spmd

    pred = np.asarray(pred, dtype=np.float32)
    box_y = np.asarray(box_y, dtype=np.int32)
    box_x = np.asarray(box_x, dtype=np.int32)
    box_h = np.asarray(box_h, dtype=np.int32)
    box_w = np.asarray(box_w, dtype=np.int32)
    box_cls = np.asarray(box_cls, dtype=np.int32)

    nc = _get_module()
    in_maps = _make_in_maps(pred, box_y, box_x, box_h, box_w)
    kw = dict(_bench) if _bench else {}
    try:
        res = run_bass_kernel_spmd(nc, in_maps, core_ids=list(range(RB * CB)), **kw)
    except Exception:
        # transient NRT/device hiccups happen; one clean retry
        res = run_bass_kernel_spmd(nc, in_maps, core_ids=list(range(RB * CB)), **kw)
    if _bench is not None:
        _CACHE["last_results"] = res
    return _finalize(res.results, box_h, box_w, box_cls)



# revision 4
# speedup vs baseline: 1.4465x; 1.4465x over previous
"""Bass/Trainium2 kernel for nn_CustomBBoxLoss (v2: fp8 + DoubleRow + box windows).

Reference computation:
    A1 = pred.sum(axis=(0,1));  A2 = (pred**2).sum(axis=(0,1))      # [H, W]
    s1[b] = sum of A1 over box b's region;  s2[b] likewise for A2
    per_box = (s2 - 2*cls*s1 + cls^2*cnt) / cnt;  loss = per_box.mean()

Each region sum is a bilinear form  s[b] = rowmask_b^T @ A @ colmask_b.
Key structural facts exploited here:
  * box_h <= 128, so a box's row mask touches <= 2 adjacent 128-row tiles.
    Sorting boxes by y makes the boxes relevant to any 512-row slab a
    CONTIGUOUS window of sorted indices (max span 87 < 128 for these
    inputs), so one <=128-wide stationary mask covers a whole core's rows
    and every data column is streamed through the PE exactly once.
  * fp8(e4m3) precision is ample for a 2e-2 tolerance (measured ~2e-4
    end-to-end), so pred streams from HBM as fp8 (1/4 the bytes) and the
    PE runs fp8 DoubleRow matmuls: the two maps of a (map-pair, row-tile)
    contract in a single pass, removing all pair-add elementwise work.

Per stream tile k=(rt,j) a combined SBUF tile holds [a | a^2 | b | b^2]
(each 1024 cols).  Squares are computed in-place by ACT/DVE/GPSIMD
(round-robin); one DoubleRow matmul per tile accumulates
  psum[box, 0:1024]    += rmask^T @ (a + b)        (s1 partials)
  psum[box, 1024:2048] += rmask^T @ (a^2 + b^2)    (s2 partials)
into a single 4-bank PSUM group over all 12 tiles.  The epilogue applies
the column mask with two fused multiply-reduce DVE ops.

Sharding: 4x2 grid (512 rows x 1024 cols per core).  Masks are built
exactly on the host (integer compares in numpy) and DMA'd: row masks
pre-replicated for DoubleRow's 2-deep k layout, col masks per core
column block.  The host sums per-core partials ("all-reduce") and
applies the closed-form per-box formula.
"""

import numpy as np
import ml_dtypes

F8 = ml_dtypes.float8_e4m3fn

H = W = 2048
B, C, N = 2, 3, 256
MAPS = B * C                      # 6
RB, CB = 4, 2                     # row-blocks x col-blocks = 8 cores
ROWS, COLS = H // RB, W // CB     # 512 x 1024 per core
P = 128                           # partitions
NRT = ROWS // P                   # 4 row tiles per core
NPAIR = MAPS // 2                 # 3 map pairs
NK = NRT * NPAIR                  # 12 streamed tiles per core
NBOX = 128                        # sorted-box window width per row slab

_CACHE = {}


def _build_module():
    import concourse.bacc as bacc
    import concourse.mybir as mybir
    import concourse.tile as tile

    f32 = mybir.dt.float32
    f8 = mybir.dt.float8e4
    Alu = mybir.AluOpType
    DR = mybir.MatmulPerfMode.DoubleRow

    nc = bacc.Bacc("TRN2", target_bir_lowering=False, debug=False)

    # tiles laid out [p, (t c)] with t in {a,b} (the two maps of a pair)
    pred_part = nc.declare_dram_parameter("pred_part", [NK, P, 2048], f8, isOutput=False)
    # row masks per row tile, replicated for DoubleRow: [p, (t box)]
    rmt = nc.declare_dram_parameter("rmt", [NRT, P, 2 * NBOX], f8, isOutput=False)
    # col mask: partition = window box index, free = col within core block
    cm = nc.declare_dram_parameter("cm", [P, COLS], f8, isOutput=False)
    out_s = nc.declare_dram_parameter("out_s", [P, 2], f32, isOutput=True)

    with tile.TileContext(nc) as tc:
        with (
            tc.tile_pool(name="persist", bufs=1) as pp,
            tc.tile_pool(name="comb", bufs=6) as comb_pool,
            tc.tile_pool(name="psum", bufs=1, space="PSUM") as psum_pool,
        ):
            # ---- small mask DMAs first (matmul 0 needs rmt[0]) ----
            rm_t = []
            for rt in range(NRT):
                t = pp.tile([P, 2 * NBOX], f8, tag=f"rm{rt}", name=f"rm{rt}")
                nc.sync.dma_start(t[:], rmt.ap()[rt])
                rm_t.append(t)
            cm_t = pp.tile([P, COLS], f8, tag="cm", name="cm")
            nc.gpsimd.dma_start(cm_t[:], cm.ap()[:])

            # ---- big streaming DMAs: [a | b] -> combined tile slots 0, 2 ----
            combs = []
            for k in range(NK):
                cb_tile = comb_pool.tile([P, 4096], f8, tag="comb", name="comb")
                dst = cb_tile[:].rearrange("p (t c) -> p t c", t=4)
                src = pred_part.ap()[k].rearrange("p (t c) -> p t c", t=2)
                nc.sync.dma_start(dst[:, 0:1, :], src[:, 0:1, :])
                nc.sync.dma_start(dst[:, 2:3, :], src[:, 1:2, :])
                combs.append(cb_tile)

            # ---- PE pipeline warm (cheap; keeps first real matmul off the
            # cold p-state) ----
            junk = pp.tile([P, 512], f8, tag="junk", name="junk")
            nc.gpsimd.memset(junk[:], 0.0)
            ps_junk = psum_pool.tile([P, 512], f32, tag="psj", name="psj")
            for _ in range(6):
                nc.tensor.matmul(ps_junk[:], junk[:, :P], junk[:],
                                 start=True, stop=True)

            # ---- main PSUM group: [box, (t1 cols | sq cols)] ----
            ps = psum_pool.tile([P, 2048], f32, tag="ps", name="ps")

            s_all = pp.tile([P, 2], f32, tag="s_all", name="s_all")

            # ---- stream: squares then one DoubleRow matmul per tile ----
            sq_engine = [nc.scalar, nc.vector, nc.scalar, nc.vector,
                         nc.scalar, nc.gpsimd, nc.scalar, nc.vector,
                         nc.scalar, nc.vector, nc.scalar, nc.gpsimd]
            for rt in range(NRT):
                for j in range(NPAIR):
                    k = rt * NPAIR + j
                    cb_tile = combs[k]
                    v4 = cb_tile[:].rearrange("p (t c) -> p t c", t=4)
                    eng = sq_engine[k]
                    if eng is nc.scalar:
                        nc.scalar.square(v4[:, 1:2, :], v4[:, 0:1, :])
                        nc.scalar.square(v4[:, 3:4, :], v4[:, 2:3, :])
                    else:
                        eng.tensor_mul(v4[:, 1:2, :], v4[:, 0:1, :], v4[:, 0:1, :])
                        eng.tensor_mul(v4[:, 3:4, :], v4[:, 2:3, :], v4[:, 2:3, :])
                    lhsT = rm_t[rt][:].rearrange("p (t b) -> p t b", t=2)
                    rhs = cb_tile[:].rearrange("p (t c) -> p t c", t=2)
                    for n in range(4):  # one matmul per PSUM bank
                        nc.tensor.matmul(
                            ps[:, n * 512:(n + 1) * 512],
                            lhsT, rhs[:, :, n * 512:(n + 1) * 512],
                            start=(k == 0), stop=(k == NK - 1), perf_mode=DR)

            # ---- epilogue: s[p] = sum_c psum[p, c] * cmask[p, c] ----
            scr = pp.tile([P, COLS], f32, tag="scr", name="scr")
            scr2 = pp.tile([P, COLS], f32, tag="scr2", name="scr2")
            nc.vector.scalar_tensor_tensor(
                out=scr[:], in0=ps[:, :COLS], scalar=1.0, in1=cm_t[:],
                op0=Alu.mult, op1=Alu.mult, accum_out=s_all[:, 0:1])
            nc.vector.scalar_tensor_tensor(
                out=scr2[:], in0=ps[:, COLS:], scalar=1.0, in1=cm_t[:],
                op0=Alu.mult, op1=Alu.mult, accum_out=s_all[:, 1:2])
            nc.gpsimd.dma_start(out_s.ap()[:], s_all[:])

    nc.compile()
    return nc


def _get_module():
    if "nc" not in _CACHE:
        _CACHE["nc"] = _build_module()
    return _CACHE["nc"]


def _plan_boxes(box_y, box_h):
    """Sort boxes by y; pick a 128-wide sorted window per row slab."""
    order = np.argsort(box_y, kind="stable")
    ys = box_y[order].astype(np.int64)
    hs = box_h[order].astype(np.int64)
    win = []
    for rb in range(RB):
        lo, hi = rb * ROWS, (rb + 1) * ROWS
        touch = np.nonzero((ys + hs > lo) & (ys < hi))[0]
        if len(touch) == 0:
            w0 = 0
        else:
            w0 = min(int(touch[0]), N - NBOX)
            assert int(touch[-1]) < w0 + NBOX, (
                f"slab {rb}: sorted-box window span {int(touch[-1]) - int(touch[0]) + 1}"
                f" exceeds {NBOX}")
        win.append(w0)
    return order, win


def _make_in_maps(pred, box_y, box_x, box_h, box_w, order, win):
    pred8 = pred.reshape(MAPS, H, W).astype(F8)
    ys = box_y[order].astype(np.int64)
    hs = box_h[order].astype(np.int64)
    xs = box_x[order].astype(np.int64)
    ws = box_w[order].astype(np.int64)

    in_maps = []
    for core in range(RB * CB):
        rb, cb = divmod(core, CB)
        slab = pred8[:, rb * ROWS:(rb + 1) * ROWS, cb * COLS:(cb + 1) * COLS]
        a = slab.reshape(NPAIR, 2, NRT, P, COLS)              # [j, t, rt, p, c]
        a = np.ascontiguousarray(a.transpose(2, 0, 3, 1, 4))  # [rt, j, p, t, c]

        w0 = win[rb]
        yw = ys[w0:w0 + NBOX]
        hw_ = hs[w0:w0 + NBOX]
        xw = xs[w0:w0 + NBOX] - cb * COLS
        ww = ws[w0:w0 + NBOX]

        # row masks: rm[rt, p, b] = yw[b] <= r < yw[b]+hw[b], r global row
        r = (rb * ROWS + np.arange(ROWS)).reshape(NRT, P, 1)
        rm = ((yw.reshape(1, 1, NBOX) <= r)
              & (r < (yw + hw_).reshape(1, 1, NBOX)))
        rmt_host = np.concatenate([rm, rm], axis=2).astype(F8)   # [rt, p, 2*NBOX]

        # col mask: cm[p, c] = xw[p] <= c < xw[p]+ww[p] (core-local cols)
        c = np.arange(COLS).reshape(1, COLS)
        cmh = ((xw.reshape(NBOX, 1) <= c)
               & (c < (xw + ww).reshape(NBOX, 1))).astype(F8)

        in_maps.append({
            "pred_part": a.reshape(NK, P, 2048),
            "rmt": np.ascontiguousarray(rmt_host),
            "cm": np.ascontiguousarray(cmh),
        })
    return in_maps


def _finalize(results, box_h, box_w, box_cls, order, win):
    s1 = np.zeros(N, np.float64)
    s2 = np.zeros(N, np.float64)
    for core, r in enumerate(results):
        rb = core // CB
        o = r["out_s"].astype(np.float64)          # [128, (s1, s2)]
        w0 = win[rb]
        s1[w0:w0 + NBOX] += o[:, 0]
        s2[w0:w0 + NBOX] += o[:, 1]
    hs = box_h[order].astype(np.float64)
    ws = box_w[order].astype(np.float64)
    cls = box_cls[order].astype(np.float64)
    cnt = float(MAPS) * hs * ws
    per_box = (s2 - 2.0 * cls * s1 + cls * cls * cnt) / cnt
    return np.asarray(per_box.mean(), dtype=np.float32)


def kernel(pred, box_y, box_x, box_h, box_w, box_cls, _bench=None):
    from concourse.bass_utils import run_bass_kernel_spmd

    pred = np.asarray(pred, dtype=np.float32)
    box_y = np.asarray(box_y, dtype=np.int32)
    box_x = np.asarray(box_x, dtype=np.int32)
    box_h = np.asarray(box_h, dtype=np.int32)
    box_w = np.asarray(box_w, dtype=np.int32)
    box_cls = np.asarray(box_cls, dtype=np.int32)

    nc = _get_module()
    order, win = _plan_boxes(box_y, box_h)
    in_maps = _make_in_maps(pred, box_y, box_x, box_h, box_w, order, win)
    kw = dict(_bench) if _bench else {}
    try:
        res = run_bass_kernel_spmd(nc, in_maps, core_ids=list(range(RB * CB)), **kw)
    except Exception:
        # transient NRT/device hiccups happen; one clean retry
        res = run_bass_kernel_spmd(nc, in_maps, core_ids=list(range(RB * CB)), **kw)
    if _bench is not None:
        _CACHE["last_results"] = res
    return _finalize(res.results, box_h, box_w, box_cls, order, win)


# revision 5
# speedup vs baseline: 1.5534x; 1.0739x over previous
"""Bass/Trainium2 kernel for nn_CustomBBoxLoss (v3: fp8 DoubleRow + box windows).

Reference computation:
    A1 = pred.sum(axis=(0,1));  A2 = (pred**2).sum(axis=(0,1))      # [H, W]
    s1[b] = sum of A1 over box b's region;  s2[b] likewise for A2
    per_box = (s2 - 2*cls*s1 + cls^2*cnt) / cnt;  loss = per_box.mean()

Each region sum is a bilinear form  s[b] = rowmask_b^T @ A @ colmask_b.
Structural facts exploited:
  * box_h <= 128, so a box's row mask touches <= 2 adjacent 128-row tiles.
    Sorting boxes by y makes the boxes relevant to any 512-row slab a
    CONTIGUOUS window of sorted indices (max span 87 < 128 here), so one
    <=128-wide stationary mask covers a whole core's rows and every data
    column streams through the PE exactly once.
  * fp8(e4m3) is ample for the 2e-2 tolerance (measured ~1e-3 end to end):
    pred streams from HBM as fp8 (1/4 the bytes) and the PE runs fp8
    DoubleRow matmuls contracting the two maps of a pair in one pass, so
    no pair-add elementwise work exists at all.

Per stream tile k=(rt,j) a combined SBUF tile holds [a | b | a^2 | b^2]
(1024 cols each; a,b = the two maps of pair j restricted to row tile rt).
The raw halves arrive as one contiguous DMA (alternating between the sync
and scalar HWDGE queues so two transfers are always in flight); squares
are computed by one fused ACT/DVE/GPSIMD op per tile (round-robin by
measured engine rates); four DoubleRow matmuls per tile accumulate
  psum[box, 0:1024]    += rmask^T (a + b)       (s1 partials)
  psum[box, 1024:2048] += rmask^T (a^2 + b^2)   (s2 partials)
into one 4-bank PSUM group over all 12 tiles.  Matmuls of the same row
tile share their stationary mask, so LDWEIGHTS is elided for all but the
first (ldweights=False).  The epilogue applies the column mask with two
fused multiply-accumulate DVE ops.

Sharding: 4x2 grid (512 rows x 1024 cols per core).  Masks are built
exactly on the host (integer compares) and DMA'd; row masks are staged
pre-replicated for DoubleRow's 2-deep K layout.  The host sums per-core
partials (the "all-reduce") and applies the closed-form per-box formula.
"""

import numpy as np
import ml_dtypes

F8 = ml_dtypes.float8_e4m3fn

H = W = 2048
B, C, N = 2, 3, 256
MAPS = B * C                      # 6
RB, CB = 4, 2                     # row-blocks x col-blocks = 8 cores
ROWS, COLS = H // RB, W // CB     # 512 x 1024 per core
P = 128                           # partitions
NRT = ROWS // P                   # 4 row tiles per core
NPAIR = MAPS // 2                 # 3 map pairs
NK = NRT * NPAIR                  # 12 streamed tiles per core
NBOX = 128                        # sorted-box window width per row slab

_CACHE = {}

# square-op engine per tile k: scalar(ACT) / vector(DVE) / gpsimd round-robin
SQ_ENGINE = ["act", "dve", "gps", "act", "dve", "gps",
             "act", "dve", "act", "dve", "act", "act"]


def _build_module():
    import concourse.bacc as bacc
    import concourse.mybir as mybir
    import concourse.tile as tile

    f32 = mybir.dt.float32
    f8 = mybir.dt.float8e4
    Alu = mybir.AluOpType
    DR = mybir.MatmulPerfMode.DoubleRow

    nc = bacc.Bacc("TRN2", target_bir_lowering=False, debug=False)

    # [a | b] per tile, contiguous 2KB rows
    pred_part = nc.declare_dram_parameter("pred_part", [NK, P, 2048], f8, isOutput=False)
    # row masks per row tile, replicated for DoubleRow: [p, (t box)]
    rmt = nc.declare_dram_parameter("rmt", [NRT, P, 2 * NBOX], f8, isOutput=False)
    cm = nc.declare_dram_parameter("cm", [P, COLS], f8, isOutput=False)
    out_s = nc.declare_dram_parameter("out_s", [P, 2], f32, isOutput=True)

    with tile.TileContext(nc) as tc:
        with (
            tc.tile_pool(name="persist", bufs=1) as pp,
            tc.tile_pool(name="comb", bufs=8) as comb_pool,
            tc.tile_pool(name="psum", bufs=1, space="PSUM") as psum_pool,
        ):
            # ---- mask DMAs on the gpsimd queue (keeps HWDGE queues clear) ----
            rm_t = []
            for rt in range(NRT):
                t = pp.tile([P, 2 * NBOX], f8, tag=f"rm{rt}", name=f"rm{rt}")
                nc.gpsimd.dma_start(t[:], rmt.ap()[rt])
                rm_t.append(t)
            cm_t = pp.tile([P, COLS], f8, tag="cm", name="cm")
            nc.gpsimd.dma_start(cm_t[:], cm.ap()[:])

            # ---- big stream: one contiguous DMA per tile, 2 HWDGE queues ----
            combs = []
            for k in range(NK):
                cb_tile = comb_pool.tile([P, 4096], f8, tag="comb", name="comb")
                q = nc.sync if k % 2 == 0 else nc.scalar
                q.dma_start(cb_tile[:, 0:2048], pred_part.ap()[k])
                combs.append(cb_tile)

            # ---- PE pipeline warm ----
            junk = pp.tile([P, 512], f8, tag="junk", name="junk")
            nc.gpsimd.memset(junk[:], 0.0)
            ps = psum_pool.tile([P, 2048], f32, tag="ps", name="ps")
            for _ in range(8):
                nc.tensor.matmul(ps[:, 0:512], junk[:, :P], junk[:],
                                 start=True, stop=True)

            s_all = pp.tile([P, 2], f32, tag="s_all", name="s_all")

            # ---- stream: fused square + 4 DoubleRow matmuls per tile ----
            for rt in range(NRT):
                for j in range(NPAIR):
                    k = rt * NPAIR + j
                    cb_tile = combs[k]
                    eng = SQ_ENGINE[k]
                    if eng == "act":
                        nc.scalar.square(cb_tile[:, 2048:4096], cb_tile[:, 0:2048])
                    elif eng == "dve":
                        nc.vector.tensor_mul(cb_tile[:, 2048:4096],
                                             cb_tile[:, 0:2048], cb_tile[:, 0:2048])
                    else:
                        nc.gpsimd.tensor_mul(cb_tile[:, 2048:4096],
                                             cb_tile[:, 0:2048], cb_tile[:, 0:2048])
                    lhsT = rm_t[rt][:].rearrange("p (t b) -> p t b", t=2)
                    v4 = cb_tile[:].rearrange("p (t c) -> p t c", t=4)
                    for q in range(2):          # 0: s1 from [a|b], 1: s2 from [sq]
                        rhs2 = v4[:, 2 * q:2 * q + 2, :]
                        for n in range(2):      # PSUM bank halves
                            mm = nc.tensor.matmul(
                                ps[:, q * 1024 + n * 512:q * 1024 + (n + 1) * 512],
                                lhsT, rhs2[:, :, n * 512:(n + 1) * 512],
                                start=(k == 0), stop=(k == NK - 1), perf_mode=DR)
                            if j != 0 or q != 0 or n != 0:
                                mm.ins.ldweights = False  # same stationary per rt

            # ---- epilogue: s[p] = sum_c psum[p, c] * cmask[p, c] ----
            scr = pp.tile([P, COLS], f32, tag="scr", name="scr")
            scr2 = pp.tile([P, COLS], f32, tag="scr2", name="scr2")
            nc.vector.scalar_tensor_tensor(
                out=scr[:], in0=ps[:, :COLS], scalar=1.0, in1=cm_t[:],
                op0=Alu.mult, op1=Alu.mult, accum_out=s_all[:, 0:1])
            nc.vector.scalar_tensor_tensor(
                out=scr2[:], in0=ps[:, COLS:], scalar=1.0, in1=cm_t[:],
                op0=Alu.mult, op1=Alu.mult, accum_out=s_all[:, 1:2])
            nc.sync.dma_start(out_s.ap()[:], s_all[:])

    nc.compile()
    return nc


def _get_module():
    if "nc" not in _CACHE:
        _CACHE["nc"] = _build_module()
    return _CACHE["nc"]


def _plan_boxes(box_y, box_h):
    """Sort boxes by y; pick a 128-wide sorted window per row slab."""
    order = np.argsort(box_y, kind="stable")
    ys = box_y[order].astype(np.int64)
    hs = box_h[order].astype(np.int64)
    win = []
    for rb in range(RB):
        lo, hi = rb * ROWS, (rb + 1) * ROWS
        touch = np.nonzero((ys + hs > lo) & (ys < hi))[0]
        if len(touch) == 0:
            w0 = 0
        else:
            w0 = min(int(touch[0]), N - NBOX)
            assert int(touch[-1]) < w0 + NBOX, (
                f"slab {rb}: sorted-box window span {int(touch[-1]) - int(touch[0]) + 1}"
                f" exceeds {NBOX}")
        win.append(w0)
    return order, win


def _make_in_maps(pred, box_y, box_x, box_h, box_w, order, win):
    pred8 = pred.reshape(MAPS, H, W).astype(F8)
    ys = box_y[order].astype(np.int64)
    hs = box_h[order].astype(np.int64)
    xs = box_x[order].astype(np.int64)
    ws = box_w[order].astype(np.int64)

    in_maps = []
    for core in range(RB * CB):
        rb, cb = divmod(core, CB)
        slab = pred8[:, rb * ROWS:(rb + 1) * ROWS, cb * COLS:(cb + 1) * COLS]
        a = slab.reshape(NPAIR, 2, NRT, P, COLS)              # [j, t, rt, p, c]
        a = np.ascontiguousarray(a.transpose(2, 0, 3, 1, 4))  # [rt, j, p, t, c]

        w0 = win[rb]
        yw = ys[w0:w0 + NBOX]
        hw_ = hs[w0:w0 + NBOX]
        xw = xs[w0:w0 + NBOX] - cb * COLS
        ww = ws[w0:w0 + NBOX]

        # row masks: rm[rt, p, b] = yw[b] <= r < yw[b]+hw[b], r global row
        r = (rb * ROWS + np.arange(ROWS)).reshape(NRT, P, 1)
        rm = ((yw.reshape(1, 1, NBOX) <= r)
              & (r < (yw + hw_).reshape(1, 1, NBOX)))
        rmt_host = np.concatenate([rm, rm], axis=2).astype(F8)   # [rt, p, 2*NBOX]

        # col mask: cm[p, c] = xw[p] <= c < xw[p]+ww[p] (core-local cols)
        c = np.arange(COLS).reshape(1, COLS)
        cmh = ((xw.reshape(NBOX, 1) <= c)
               & (c < (xw + ww).reshape(NBOX, 1))).astype(F8)

        in_maps.append({
            "pred_part": a.reshape(NK, P, 2048),
            "rmt": np.ascontiguousarray(rmt_host),
            "cm": np.ascontiguousarray(cmh),
        })
    return in_maps


def _finalize(results, box_h, box_w, box_cls, order, win):
    s1 = np.zeros(N, np.float64)
    s2 = np.zeros(N, np.float64)
    for core, r in enumerate(results):
        rb = core // CB
        o = r["out_s"].astype(np.float64)          # [128, (s1, s2)]
        w0 = win[rb]
        s1[w0:w0 + NBOX] += o[:, 0]
        s2[w0:w0 + NBOX] += o[:, 1]
    hs = box_h[order].astype(np.float64)
    ws = box_w[order].astype(np.float64)
    cls = box_cls[order].astype(np.float64)
    cnt = float(MAPS) * hs * ws
    per_box = (s2 - 2.0 * cls * s1 + cls * cls * cnt) / cnt
    return np.asarray(per_box.mean(), dtype=np.float32)


def kernel(pred, box_y, box_x, box_h, box_w, box_cls, _bench=None):
    from concourse.bass_utils import run_bass_kernel_spmd

    pred = np.asarray(pred, dtype=np.float32)
    box_y = np.asarray(box_y, dtype=np.int32)
    box_x = np.asarray(box_x, dtype=np.int32)
    box_h = np.asarray(box_h, dtype=np.int32)
    box_w = np.asarray(box_w, dtype=np.int32)
    box_cls = np.asarray(box_cls, dtype=np.int32)

    nc = _get_module()
    order, win = _plan_boxes(box_y, box_h)
    in_maps = _make_in_maps(pred, box_y, box_x, box_h, box_w, order, win)
    kw = dict(_bench) if _bench else {}
    try:
        res = run_bass_kernel_spmd(nc, in_maps, core_ids=list(range(RB * CB)), **kw)
    except Exception:
        # transient NRT/device hiccups happen; one clean retry
        res = run_bass_kernel_spmd(nc, in_maps, core_ids=list(range(RB * CB)), **kw)
    if _bench is not None:
        _CACHE["last_results"] = res
    return _finalize(res.results, box_h, box_w, box_cls, order, win)
